# revision 11
# baseline (speedup 1.0000x reference)
"""RWKV time-mix (WKV) kernel for 8 Trainium2 NeuronCores.

Strategy
--------
Data-parallel over B: each of the 8 cores gets 8 batches. Per core/batch,
everything runs in channel-major layout [C(part), T(free)]:

  host:   x^T (bf16), W^T (bf16), per-channel constants precomputed
  chip:   time-shift via free-dim slice, mixes on DVE (bf16),
          k/v/r projections on TensorE (bf16 -> f32 PSUM),
          WKV recurrence via DVE tensor_tensor_scan (f32):
              A_t = D*A_{t-1} + exp(k_t)*v_t     (scan, per channel)
              B_t = D*B_{t-1} + exp(k_t)
              y_t = (A_t + (e^u - 1)*EV_t) / (B_t + (e^u - 1)*E_t)
          division + sigmoid via exp/ln on ScalarE (single ACT table set):
              rwkv = num * exp(-(ln(den) + ln(1 + exp(-r))))
          output projection on TensorE, DMA out as out^T (f32)
  host:   concat + transpose back
"""

import contextlib
import ctypes
import os
import sys
import types

import numpy as np
import ml_dtypes


def _ensure_ntff_hook():
    """The image's antenv package lacks axon_hooks; provide it (and a
    working ctypes NTFF profile hook) so trace=True paths don't crash."""
    try:
        import antenv.axon_hooks  # noqa: F401
        return
    except ImportError:
        pass
    try:
        import antenv
    except ImportError:
        antenv = types.ModuleType("antenv")
        sys.modules["antenv"] = antenv
    mod = types.ModuleType("antenv.axon_hooks")
    _hook = [None]
    mod.set_axon_ntff_profile_hook = lambda h: _hook.__setitem__(0, h)
    mod.get_axon_ntff_profile_hook = lambda: _hook[0]
    sys.modules["antenv.axon_hooks"] = mod
    sys.modules["antenv"].axon_hooks = mod

    so_path = "/opt/axon/libaxon_pjrt.so"
    if os.path.exists(so_path):
        try:
            lib = ctypes.CDLL(so_path)
            if hasattr(lib, "axon_start_nrt_profile"):
                lib.axon_start_nrt_profile.argtypes = [
                    ctypes.POINTER(ctypes.c_int64), ctypes.c_size_t]
                lib.axon_start_nrt_profile.restype = ctypes.c_int64
                lib.axon_stop_nrt_profile.argtypes = [ctypes.c_char_p]
                lib.axon_stop_nrt_profile.restype = ctypes.c_int64

                @contextlib.contextmanager
                def _profile(output_dir, device_ids):
                    import jax
                    jax.devices()
                    if device_ids:
                        ids = (ctypes.c_int64 * len(device_ids))(*device_ids)
                        rc = lib.axon_start_nrt_profile(ids, len(device_ids))
                    else:
                        rc = lib.axon_start_nrt_profile(None, 0)
                    if rc != 0:
                        raise RuntimeError(f"axon_start_nrt_profile rc={rc}")
                    try:
                        yield
                    finally:
                        n = lib.axon_stop_nrt_profile(str(output_dir).encode())
                        print(f"profile: {n} file(s) written to {output_dir}",
                              file=sys.stderr)

                mod.set_axon_ntff_profile_hook(_profile)
        except OSError:
            pass


_ensure_ntff_hook()

import concourse.bass as bass
import concourse.mybir as mybir
import concourse.tile as tile
from concourse import bacc
from concourse.bass_utils import run_bass_kernel_spmd

B, T, C = 64, 512, 1024
NCORES = 8
BPC = B // NCORES          # batches per core
P = 128
CT = C // P                # channel tiles

F32 = mybir.dt.float32
BF16 = mybir.dt.bfloat16
AF = mybir.ActivationFunctionType
OP = mybir.AluOpType

_nc_cache = {}


def build_nc(debug=False):
    nc = bacc.Bacc()
    dbg_names = ("xb", "xk", "E", "EV0", "A", "Bs", "num", "den", "er", "lnr", "f", "rw0")
    dbg = {}
    if debug:
        for n in dbg_names:
            dbg[n] = nc.declare_dram_parameter(f"dbg_{n}", [P, T], F32, isOutput=True)

    xt = nc.declare_dram_parameter("xt", [BPC, C, T], BF16, isOutput=False)
    wk = nc.declare_dram_parameter("wk", [C, C], BF16, isOutput=False)
    wv = nc.declare_dram_parameter("wv", [C, C], BF16, isOutput=False)
    wr = nc.declare_dram_parameter("wr", [C, C], BF16, isOutput=False)
    wo = nc.declare_dram_parameter("wo", [C, C], BF16, isOutput=False)
    # per-channel constants [P, CT, 5]: tmk, tmv, tmr, eu_m1, D
    cvec = nc.declare_dram_parameter("cvec", [P, CT, 5], F32, isOutput=False)
    out = nc.declare_dram_parameter("out", [BPC, C, T], F32, isOutput=True)

    with tile.TileContext(nc) as tc:
        with (
            tc.tile_pool(name="singles", bufs=1) as singles,
            tc.tile_pool(name="xbp", bufs=2) as xbp,
            tc.tile_pool(name="mixp", bufs=2) as mixp,
            tc.tile_pool(name="stagec", bufs=2) as stagec,
            tc.tile_pool(name="rwkvp", bufs=2) as rwkvp,
            tc.tile_pool(name="outp", bufs=3) as outp,
            tc.tile_pool(name="ps_kvr", bufs=2, space="PSUM") as ps_kvr,
            tc.tile_pool(name="ps_out", bufs=2, space="PSUM") as ps_out,
        ):
            # ---- one-time loads ----
            w_sb = {}
            for name, par in (("k", wk), ("v", wv), ("r", wr), ("o", wo)):
                t = singles.tile([P, CT, C], BF16, tag=f"w{name}", name=f"w{name}")
                nc.sync.dma_start(out=t[:], in_=par.rearrange("(ct p) d -> p ct d", p=P))
                w_sb[name] = t
            cv = singles.tile([P, CT, 5], F32, tag="cvec")
            nc.sync.dma_start(out=cv[:], in_=cvec[:])

            # D broadcast tiles for the scan multiplier
            Db = singles.tile([P, CT, T], F32, tag="Db")
            nc.vector.memset(Db[:], 1.0)
            for j in range(CT):
                nc.vector.tensor_scalar_mul(Db[:, j, :], Db[:, j, :], cv[:, j, 4:5])

            def emit_out_proj(b, rw):
                for dj in range(CT):
                    pso = ps_out.tile([P, T], F32, tag="pso")
                    for kt in range(CT):
                        nc.tensor.matmul(
                            pso[:],
                            w_sb["o"][:, kt, dj * P:(dj + 1) * P],
                            rw[:, kt, :],
                            start=(kt == 0),
                            stop=(kt == CT - 1),
                        )
                    osb = outp.tile([P, T], F32, tag="osb")
                    nc.scalar.copy(osb[:], pso[:])
                    nc.sync.dma_start(
                        out=out[b].rearrange("(ct p) t -> p ct t", p=P)[:, dj, :],
                        in_=osb[:],
                    )

            prev = None  # (b, rwkv tile) pending output projection
            for b in range(BPC):
                # ---- stage A: load x^T with a zero guard column, mix ----
                xb = xbp.tile([P, CT, T + 1], BF16, tag="xb")
                nc.vector.memset(xb[:, :, 0:1], 0.0)
                nc.sync.dma_start(
                    out=xb[:, :, 1:T + 1],
                    in_=xt[b].rearrange("(ct p) t -> p ct t", p=P),
                )
                mix = {}
                for name in ("k", "v", "r"):
                    mix[name] = mixp.tile([P, CT, T], BF16, tag=f"x{name}", name=f"x{name}")
                for j in range(CT):
                    x_ap = xb[:, j, 1:T + 1]
                    xx_ap = xb[:, j, 0:T]
                    dif = mixp.tile([P, T], BF16, tag="dif")
                    nc.vector.tensor_tensor(dif[:], x_ap, xx_ap, OP.subtract)
                    for ci, name in enumerate(("k", "v", "r")):
                        m = mix[name][:, j, :]
                        nc.vector.tensor_scalar_mul(m, dif[:], cv[:, j, ci:ci + 1])
                        nc.vector.tensor_tensor(m, m, xx_ap, OP.add)

                # ---- stage B+C: projections + WKV per channel-tile ----
                rw = rwkvp.tile([P, CT, T], BF16, tag="rwkv")
                for j in range(CT):
                    psk = ps_kvr.tile([P, T], F32, tag="psk")
                    psv = ps_kvr.tile([P, T], F32, tag="psv")
                    psr = ps_kvr.tile([P, T], F32, tag="psr")
                    for nm, ps in (("k", psk), ("v", psv), ("r", psr)):
                        for kt in range(CT):
                            nc.tensor.matmul(
                                ps[:],
                                w_sb[nm][:, kt, j * P:(j + 1) * P],
                                mix[nm][:, kt, :],
                                start=(kt == 0),
                                stop=(kt == CT - 1),
                            )

                    eu = cv[:, j, 3:4]
                    E = stagec.tile([P, T], F32, tag="E")
                    nc.scalar.activation(E[:], psk[:], AF.Exp)
                    EV = stagec.tile([P, T], F32, tag="EV")
                    nc.vector.tensor_tensor(EV[:], E[:], psv[:], OP.mult)

                    # exclusive scans: A[:, t] = sum_{i<t} D^(t-1-i) EV_i, A[:, 0] = 0
                    A = stagec.tile([P, T], F32, tag="A")
                    nc.gpsimd.memset(A[:, 0:1], 0.0)
                    nc.vector.tensor_tensor_scan(
                        A[:, 1:T], Db[:, j, 0:T - 1], EV[:, 0:T - 1], 0.0, OP.mult, OP.add)
                    Bs = stagec.tile([P, T], F32, tag="Bs")
                    nc.gpsimd.memset(Bs[:, 0:1], 0.0)
                    nc.vector.tensor_tensor_scan(
                        Bs[:, 1:T], Db[:, j, 0:T - 1], E[:, 0:T - 1], 0.0, OP.mult, OP.add)

                    # num -> EV slot, den -> E slot (in-place)
                    nc.vector.scalar_tensor_tensor(EV[:], EV[:], eu, A[:], OP.mult, OP.add)
                    nc.vector.scalar_tensor_tensor(E[:], E[:], eu, Bs[:], OP.mult, OP.add)

                    # sigmoid(r): ln(1 + exp(-r)); division: ln(den)
                    er = stagec.tile([P, T], F32, tag="er")
                    nc.scalar.activation(er[:], psr[:], AF.Exp, scale=-1.0)
                    lnr = stagec.tile([P, T], F32, tag="lnr")
                    nc.scalar.activation(lnr[:], er[:], AF.Ln, bias=1.0)
                    ld = stagec.tile([P, T], F32, tag="ld")
                    nc.scalar.activation(ld[:], E[:], AF.Ln)
                    nc.vector.tensor_tensor(lnr[:], lnr[:], ld[:], OP.add)
                    f = stagec.tile([P, T], F32, tag="f")
                    nc.scalar.activation(f[:], lnr[:], AF.Exp, scale=-1.0)
                    nc.vector.tensor_tensor(rw[:, j, :], EV[:], f[:], OP.mult)

                    if debug and b == 0 and j == 0:
                        def dump(name, ap, cast=False):
                            if cast:
                                tmp = stagec.tile([P, T], F32, tag="dbgtmp", name="dbgtmp")
                                nc.vector.tensor_copy(tmp[:], ap)
                                ap = tmp[:]
                            nc.sync.dma_start(out=dbg[name][:], in_=ap)
                        dump("xb", xb[:, 0, 1:T + 1], cast=True)
                        dump("xk", mix["k"][:, 0, :], cast=True)
                        dump("E", E[:])      # den after in-place
                        dump("EV0", EV[:])   # num after in-place
                        dump("A", A[:])
                        dump("Bs", Bs[:])
                        dump("num", EV[:])
                        dump("den", E[:])
                        dump("er", er[:])
                        dump("lnr", lnr[:])
                        dump("f", f[:])
                        dump("rw0", rw[:, 0, :], cast=True)

                if prev is not None:
                    emit_out_proj(*prev)
                prev = (b, rw)
            emit_out_proj(*prev)

    nc.compile()
    return nc


def _host_prep(x, time_decay, time_first, time_mix_k, time_mix_v, time_mix_r,
               Wk, Wv, Wr, Wo):
    bf = ml_dtypes.bfloat16
    f32 = np.float32
    xt = np.ascontiguousarray(x.transpose(0, 2, 1)).astype(bf)      # [B, C, T]
    wkt = np.ascontiguousarray(np.asarray(Wk, f32).T).astype(bf)    # [c, d]
    wvt = np.ascontiguousarray(np.asarray(Wv, f32).T).astype(bf)
    wrt = np.ascontiguousarray(np.asarray(Wr, f32).T).astype(bf)
    wot = np.ascontiguousarray(np.asarray(Wo, f32).T).astype(bf)

    D = np.exp(-np.exp(np.asarray(time_decay, f32))).astype(f32)
    eu_m1 = np.exp(np.asarray(time_first, f32)).astype(f32)  # e^u (col 3)
    tmk = np.asarray(time_mix_k, f32).reshape(C)
    tmv = np.asarray(time_mix_v, f32).reshape(C)
    tmr = np.asarray(time_mix_r, f32).reshape(C)
    cvec = np.stack([tmk, tmv, tmr, eu_m1, D], axis=-1)             # [C, 5]
    cvec = np.ascontiguousarray(cvec.reshape(CT, P, 5).transpose(1, 0, 2)).astype(f32)

    in_maps = []
    for i in range(NCORES):
        in_maps.append({
            "xt": xt[i * BPC:(i + 1) * BPC],
            "wk": wkt, "wv": wvt, "wr": wrt, "wo": wot,
            "cvec": cvec,
        })
    return in_maps


def kernel(x, time_decay, time_first, time_mix_k, time_mix_v, time_mix_r,
           Wk, Wv, Wr, Wo):
    x = np.asarray(x, np.float32)
    in_maps = _host_prep(x, time_decay, time_first, time_mix_k, time_mix_v,
                         time_mix_r, Wk, Wv, Wr, Wo)
    if "nc" not in _nc_cache:
        _nc_cache["nc"] = build_nc()
    res = run_bass_kernel_spmd(_nc_cache["nc"], in_maps, core_ids=list(range(NCORES)))
    _nc_cache["last_results"] = res
    full = np.concatenate([res.results[i]["out"] for i in range(NCORES)], axis=0)
    return np.ascontiguousarray(full.transpose(0, 2, 1))
